# revision 1
# baseline (speedup 1.0000x reference)
"""Trainium2 Bass kernel for the capsule-routing layer (nn_Caps_Layer).

Full inputs: x [32, 512, 768] f32, W [1, 768, 512] f32.
Output: [32, 16, 32] f32.

Strategy: data-parallel over batch across 8 NeuronCores (4 batches/core).
Per core:
  - load x shard + W, transpose x on the PE (fp32r) to get xT [d, s]
  - u_hat  u [s, (n c)] tiles via PE matmuls (fp32r, 1 cyc/row at N=512)
  - uT [(n c), s] tiles via PE transposes of u
  - 3 routing iterations entirely on-chip:
      c = softmax_n(b)  (b=0 on iter 0 -> ones weights; squash is
      scale-invariant so the 1/16 factor drops out)
      outputs_raw[n,:] = sum_s c[s,n] * u[s, (n,:)]   (PE, block-diag extract
      via [16,512] matmul + PE transpose + mask)
      Mblk = outputs/||outputs||  scattered block-diag [(n c), n]
      b = Mblk.T @ uT  (PE)
  - final outputs gathered via a constant comb matrix and DMA'd out.
"""
import numpy as np
import concourse.bass as bass
import concourse.mybir as mybir
import concourse.tile as tile
from concourse import bacc
from concourse.bass import ts, ds
from concourse.bass_utils import run_bass_kernel_spmd
from concourse.tile import add_dep_helper


def _chain(insts):
    """Pin issue order of a matmul accumulation group (scheduling-only edges)."""
    for a, b in zip(insts[1:], insts[:-1]):
        add_dep_helper(a.ins, b.ins, sync=False, reason="mm group order")

F32 = mybir.dt.float32
F32R = mybir.dt.float32r
AF = mybir.ActivationFunctionType
AX = mybir.AxisListType
OP = mybir.AluOpType

NCORES = 8
B, S, D = 32, 512, 768
N, C = 16, 32
NC = N * C            # 512
BL = B // NCORES      # 4 batches per core
EPS = 1e-7
SCN = S // 128        # 4 s-chunks
DCN = D // 128        # 6 d-chunks
KCN = NC // 128       # 4 nc-chunks
ROUTINGS = 3


def _build_module():
    nc = bacc.Bacc("TRN2", target_bir_lowering=False, num_devices=NCORES)
    X = nc.dram_tensor("x", [BL, S, D], F32R, kind="ExternalInput")
    W = nc.dram_tensor("w", [D, NC], F32R, kind="ExternalInput")
    IDR = nc.dram_tensor("identr", [128, 128], F32R, kind="ExternalInput")
    ID16 = nc.dram_tensor("ident16", [128, 16], F32, kind="ExternalInput")
    CW0 = nc.dram_tensor("cw0", [128, 16], F32R, kind="ExternalInput")
    MASKS = nc.dram_tensor("masks", [128, 64], F32, kind="ExternalInput")
    GM = nc.dram_tensor("gmat", [128, 32], F32, kind="ExternalInput")
    O128 = nc.dram_tensor("ones128", [128, 1], F32, kind="ExternalInput")
    O1X = nc.dram_tensor("ones1x128", [1, 128], F32, kind="ExternalInput")
    OUT = nc.dram_tensor("out", [BL, N, C], F32, kind="ExternalOutput")

    cp_flip = [0]

    with tile.TileContext(nc) as tc:
        with (
            tc.tile_pool(name="const", bufs=1) as pc,
            tc.tile_pool(name="xp", bufs=3) as px_pool,
            tc.tile_pool(name="xtp", bufs=24) as pxt_pool,
            tc.tile_pool(name="up", bufs=16) as pu_pool,
            tc.tile_pool(name="utp", bufs=16) as put_pool,
            tc.tile_pool(name="rt", bufs=3) as prt,
            tc.tile_pool(name="mmp", bufs=3, space="PSUM") as pmm,
            tc.tile_pool(name="trp", bufs=3, space="PSUM") as ptr,
            tc.tile_pool(name="smp", bufs=2, space="PSUM") as psm,
        ):
            def cp(dst, src):
                # PSUM->SBUF copies: 1/3 DVE, 2/3 ACT (DVE pays a per-op DRAIN)
                if cp_flip[0] % 3 == 0:
                    nc.vector.tensor_copy(dst, src)
                else:
                    nc.scalar.copy(dst, src)
                cp_flip[0] += 1

            # ---- constants ----
            identr = pc.tile([128, 128], F32R, tag="identr")
            ident16 = pc.tile([128, 16], F32, tag="ident16")
            cw0 = pc.tile([128, 16], F32R, tag="cw0")
            masks = pc.tile([128, 64], F32, tag="masks")
            gmat = pc.tile([128, 32], F32, tag="gmat")
            ones128 = pc.tile([128, 1], F32, tag="ones128")
            ones1x = pc.tile([1, 128], F32, tag="ones1x")
            wsb = pc.tile([128, DCN, NC], F32R, tag="w")
            epst = pc.tile([1, 1], F32, tag="eps")
            nc.vector.memset(epst[:], EPS)

            def prefetch_act(func):
                # dummy [1,1] activation: hoists the ACT table-set load off
                # the routing latency chain (the real op then loads nothing)
                dum = prt.tile([1, 1], F32, tag="dum")
                nc.scalar.activation(dum[:], epst[:], func)
            # consts + W ride the ACT HWDGE ring so x loads start immediately
            # on the SP ring; identr (needed by the first transposes) and W
            # (needed by the first u-matmuls) go first.
            nc.scalar.dma_start(identr[:], IDR[:, :])
            nc.scalar.dma_start(
                wsb[:, 0:3, :],
                W[0:384, :].rearrange("(dc p) n -> p dc n", p=128),
            )
            nc.scalar.dma_start(
                wsb[:, 3:6, :],
                W[384:768, :].rearrange("(dc p) n -> p dc n", p=128),
            )
            # ---- stage A: u and uT per batch ----
            us = [[None] * SCN for _ in range(BL)]
            uts = [[None] * KCN for _ in range(BL)]
            for b in range(BL):
                xband = px_pool.tile([128, SCN, D], F32R, tag="x")
                if b == 0:
                    # split the first load so the PE can start after ~400KB
                    for sc in range(SCN):
                        nc.sync.dma_start(
                            xband[:, sc, :],
                            X[b, ds(sc * 128, 128), :],
                        )
                else:
                    # batch 1 rides the ACT ring (behind W) so it lands well
                    # before the SP ring finishes batch 0's four chunks
                    eng = nc.scalar if b == 1 else nc.sync
                    eng.dma_start(
                        xband[:], X[b, :, :].rearrange("(sc p) d -> p sc d", p=128)
                    )
                def emit_umm(sc, xt_of_dc):
                    pu = pmm.tile([128, 512], F32, tag="mm")
                    for dc in range(DCN):
                        nc.tensor.matmul(
                            pu[:],
                            xt_of_dc[dc][:, ts(sc, 128)],
                            wsb[:, dc, :],
                            start=(dc == 0),
                            stop=(dc == DCN - 1),
                        )
                    u = pu_pool.tile([128, 512], F32R, tag="u")
                    cp(u[:], pu[:])
                    us[b][sc] = u

                if False:
                    # (tried: s-chunk-major pipeline for batch 0 — the extra
                    # per-chunk evacuation overhead outweighed the gap win)
                    xts = [pxt_pool.tile([128, 512], F32R, tag="xt",
                                         name=f"xt0_{dc}")
                           for dc in range(DCN)]
                    for sc in range(SCN):
                        for dc in range(DCN):
                            pxt = ptr.tile([128, 128], F32R, tag="tr")
                            nc.tensor.transpose(
                                pxt[:],
                                xband[:, sc, ds(dc * 128, 128)],
                                identr[:],
                            )
                            if dc % 2 == 0:
                                nc.vector.tensor_copy(
                                    xts[dc][:, ts(sc, 128)], pxt[:])
                            else:
                                nc.scalar.copy(
                                    xts[dc][:, ts(sc, 128)], pxt[:])
                        emit_umm(sc, xts)
                else:
                    # xT tiles: [128(d-chunk), 512(s)]
                    xts = []
                    for dc in range(DCN):
                        pxt = ptr.tile([128, 512], F32R, tag="tr")
                        for sc in range(SCN):
                            nc.tensor.transpose(
                                pxt[:, ts(sc, 128)],
                                xband[:, sc, ds(dc * 128, 128)],
                                identr[:],
                            )
                        xt = pxt_pool.tile([128, 512], F32R, tag="xt")
                        if b < 2:
                            nc.vector.tensor_copy(xt[:, 0:256], pxt[:, 0:256])
                            nc.scalar.copy(xt[:, 256:512], pxt[:, 256:512])
                        else:
                            cp(xt[:], pxt[:])
                        xts.append(xt)
                    for sc in range(SCN):
                        emit_umm(sc, xts)
                # uT tiles: [128(nc-chunk), 512(s)]
                for kc in range(KCN):
                    put = ptr.tile([128, 512], F32R, tag="tr")
                    for sc in range(SCN):
                        nc.tensor.transpose(
                            put[:, ts(sc, 128)],
                            us[b][sc][:, ts(kc, 128)],
                            identr[:],
                        )
                    ut = put_pool.tile([128, 512], F32R, tag="ut")
                    cp(ut[:], put[:])
                    uts[b][kc] = ut
                if b == 0:
                    prefetch_act(AF.Sqrt)

            # small constants are only needed from the routing phase onward
            nc.scalar.dma_start(ident16[:], ID16[:, :])
            nc.scalar.dma_start(cw0[:], CW0[:, :])
            nc.scalar.dma_start(masks[:], MASKS[:, :])
            nc.scalar.dma_start(gmat[:], GM[:, :])
            nc.scalar.dma_start(ones128[:], O128[:, :])
            nc.scalar.dma_start(ones1x[:], O1X[:, :])

            # ---- routing ----
            # Softmax + squash post-processing are pipelined per batch
            # (quartered) so each chain's DVE/ACT latency is covered by the
            # other batches' PE matmuls.
            cw = None     # [128, (b sc n) = 256] F32R
            mblk = None   # [128, (b k n) = 256] F32R (normalized, block diag)
            for it in range(ROUTINGS):
                last = it == ROUTINGS - 1
                if it > 0:
                    pbt = ptr.tile([128, 256], F32, tag="tr")
                    expb = prt.tile([128, 256], F32, tag="expb")
                    zsum = prt.tile([128, 16], F32, tag="zsum")
                    zrec = prt.tile([128, 16], F32, tag="zrec")
                    cw = prt.tile([128, 256], F32R, tag="cw")
                    for b in range(BL):
                        pbn = pmm.tile([16, 512], F32, tag="mm")
                        for kc in range(KCN):
                            nc.tensor.matmul(
                                pbn[:],
                                mblk[:, ds(b * 64 + kc * 16, 16)],
                                uts[b][kc][:],
                                start=(kc == 0),
                                stop=(kc == KCN - 1),
                            )
                        bsb = prt.tile([16, 512], F32, tag="bsb")
                        cp(bsb[:], pbn[:])
                        # transpose b to [128(s), (b sc n)]
                        for sc in range(SCN):
                            nc.tensor.transpose(
                                pbt[:, ds(b * 64 + sc * 16, 16)],
                                bsb[:, ts(sc, 128)],
                                ident16[0:16, :],
                            )
                        # softmax over n (free axis, groups of 16)
                        hs = ds(64 * b, 64)
                        nc.scalar.activation(expb[:, hs], pbt[:, hs], AF.Exp)
                        gsl = ds(4 * b, 4)
                        nc.vector.tensor_reduce(
                            zsum[:, gsl],
                            expb[:, hs].rearrange("p (g n) -> p g n", g=4),
                            axis=AX.X,
                            op=OP.add,
                        )
                        nc.vector.reciprocal(zrec[:, gsl], zsum[:, gsl])
                        zr_ap = zrec[:, gsl]
                        zr_b = bass.AP(
                            tensor=zr_ap.tensor,
                            offset=zr_ap.offset,
                            ap=[zr_ap.ap[0], [1, 4], [0, 16]],
                        )
                        nc.vector.tensor_mul(
                            cw[:, hs].rearrange("p (g n) -> p g n", g=4),
                            expb[:, hs].rearrange("p (g n) -> p g n", g=4),
                            zr_b,
                        )
                    prefetch_act(AF.Sqrt)

                # outputs einsum + squash postproc, per batch
                pot = ptr.tile([128, 256], F32, tag="tr")
                mraw = prt.tile([128, 256], F32, tag="mraw")
                sq = prt.tile([128, 256], F32, tag="sq")
                pnsq = psm.tile([1, 256], F32, tag="sm")
                nred = prt.tile([1, 64], F32, tag="nred")
                sqt = prt.tile([1, 64], F32, tag="sqt")
                invn = prt.tile([1, 64], F32, tag="invn")
                pinv = psm.tile([128, 64], F32, tag="sm")
                if last:
                    fin = prt.tile([32, 64], F32, tag="fin")
                    invb = prt.tile([32, 64], F32, tag="invb")
                else:
                    mblk = prt.tile([128, 256], F32R, tag="mblk")
                for b in range(BL):
                    if it == 0:
                        # uniform weights: out_full rows are identical, so
                        # compute just the column-sum row usum = 1.T @ u
                        pof = pmm.tile([1, 512], F32, tag="mm", name=f"pus{b}")
                        for sc in range(SCN):
                            nc.tensor.matmul(
                                pof[:],
                                cw0[:, 0:1],
                                us[b][sc][:],
                                start=(sc == 0),
                                stop=(sc == SCN - 1),
                            )
                        ofsb = prt.tile([1, 512], F32, tag="of", name=f"us{b}")
                        cp(ofsb[:], pof[:])
                        # scatter usum chunks to partitions: [1,128] -> [128,1]
                        for kc in range(KCN):
                            nc.tensor.transpose(
                                pot[:, ds(b * 64 + kc * 16, 1)],
                                ofsb[:, ts(kc, 128)],
                                ident16[0:1, 0:1],
                            )
                        hs = ds(64 * b, 64)
                        # mask * usum broadcast across the 16 capsule columns
                        p_ap = pot[:, ds(b * 64, 64)]
                        p_b = bass.AP(
                            tensor=p_ap.tensor,
                            offset=p_ap.offset,
                            ap=[p_ap.ap[0], [16, 4], [0, 16]],
                        )
                        nc.vector.tensor_mul(
                            mraw[:, hs].rearrange("p (k n) -> p k n", k=4),
                            masks[:].rearrange("p (k n) -> p k n", k=4),
                            p_b,
                        )
                    else:
                        pof = pmm.tile([16, 512], F32, tag="mm", name=f"pof{b}")
                        for sc in range(SCN):
                            cw_ap = cw[:, ds(b * 64 + sc * 16, 16)]
                            nc.tensor.matmul(
                                pof[:],
                                cw_ap,
                                us[b][sc][:],
                                start=(sc == 0),
                                stop=(sc == SCN - 1),
                            )
                        ofsb = prt.tile([16, 512], F32, tag="of", name=f"of{b}")
                        cp(ofsb[:], pof[:])
                        # transpose out_full chunks -> [128(nc), (b k n)]
                        for kc in range(KCN):
                            nc.tensor.transpose(
                                pot[:, ds(b * 64 + kc * 16, 16)],
                                ofsb[:, ts(kc, 128)],
                                ident16[0:16, :],
                            )
                        hs = ds(64 * b, 64)
                        # mask to block-diagonal
                        nc.vector.tensor_mul(mraw[:, hs], pot[:, hs], masks[:])
                    # squared norms -> [1, (k n)]
                    nc.vector.tensor_mul(sq[:, hs], mraw[:, hs], mraw[:, hs])
                    nc.tensor.matmul(
                        pnsq[:, ts(b, 64)],
                        ones128[:],
                        sq[:, hs],
                        start=True,
                        stop=True,
                    )
                    nsl = ds(16 * b, 16)
                    nc.vector.tensor_reduce(
                        nred[:, nsl],
                        pnsq[:, hs].rearrange("o (k n) -> o n k", k=4),
                        axis=AX.X,
                        op=OP.add,
                    )
                    # invnorm = 1/sqrt(nsq + eps), broadcast to all partitions
                    nc.scalar.activation(sqt[:, nsl], nred[:, nsl], AF.Sqrt,
                                         bias=epst[:])
                    nc.vector.reciprocal(invn[:, nsl], sqt[:, nsl])
                    nc.tensor.matmul(pinv[:, nsl], ones1x[:], invn[:, nsl],
                                     start=True, stop=True)
                    if not last:
                        # Mblk = mraw * invnorm (per n column group)
                        pv = pinv[:, nsl]
                        inv_b = bass.AP(
                            tensor=pv.tensor,
                            offset=pv.offset,
                            ap=[pv.ap[0], [0, 4], [1, 16]],
                        )
                        nc.vector.tensor_mul(
                            mblk[:, hs].rearrange("p (k n) -> p k n", k=4),
                            mraw[:, hs].rearrange("p (k n) -> p k n", k=4),
                            inv_b,
                        )
                    else:
                        # final gather straight off the unnormalized mraw: the
                        # G-matmuls overlap the sqrt/invnorm chain on the PE;
                        # normalization is applied on the [32, 16] result.
                        nc.vector.tensor_copy(invb[:, nsl], pinv[0:32, nsl])
                        pf = psm.tile([32, 16], F32, tag="sm")
                        for kc in range(KCN):
                            nc.tensor.matmul(
                                pf[:],
                                gmat[:],
                                mraw[:, ds(b * 64 + kc * 16, 16)],
                                start=(kc == 0),
                                stop=(kc == KCN - 1),
                            )
                        nc.vector.tensor_mul(
                            fin[:, ts(b, 16)],
                            pf[:],
                            invb[:, nsl],
                        )
                        out_eng = nc.sync if b % 2 == 0 else nc.scalar
                        out_eng.dma_start(
                            OUT[b, :, :].rearrange("n c -> c n"),
                            fin[:, ts(b, 16)],
                        )
                if it < ROUTINGS - 1:
                    prefetch_act(AF.Exp)

    nc.compile()
    return nc


def _make_consts():
    identr = np.eye(128, dtype=np.float32)
    ident16 = np.zeros((128, 16), dtype=np.float32)
    for b in range(4):
        ident16[32 * b:32 * b + 16, :] = np.eye(16, dtype=np.float32)
    cw0 = np.ones((128, 16), dtype=np.float32)
    masks = np.zeros((128, 64), dtype=np.float32)
    for k in range(4):
        for g in range(4):
            n = 4 * k + g
            masks[32 * g:32 * (g + 1), 16 * k + n] = 1.0
    gmat = np.tile(np.eye(32, dtype=np.float32), (4, 1))
    ones128 = np.ones((128, 1), dtype=np.float32)
    ones1x = np.ones((1, 128), dtype=np.float32)
    return {
        "identr": identr, "ident16": ident16, "cw0": cw0, "masks": masks,
        "gmat": gmat, "ones128": ones128, "ones1x128": ones1x,
    }


_NC_CACHE = []


def kernel(x: np.ndarray, W: np.ndarray) -> np.ndarray:
    assert x.shape == (B, S, D) and W.shape == (1, D, NC)
    if not _NC_CACHE:
        _NC_CACHE.append(_build_module())
    nc = _NC_CACHE[0]
    consts = _make_consts()
    w2 = np.ascontiguousarray(W[0], dtype=np.float32)
    in_maps = []
    for i in range(NCORES):
        m = dict(consts)
        m["x"] = np.ascontiguousarray(x[i * BL:(i + 1) * BL], dtype=np.float32)
        m["w"] = w2
        in_maps.append(m)
    res = run_bass_kernel_spmd(nc, in_maps, list(range(NCORES)))
    out = np.concatenate([res.results[i]["out"] for i in range(NCORES)], axis=0)
    return out.astype(np.float32)



# revision 4
# speedup vs baseline: 1.7557x; 1.7557x over previous
"""Trainium2 Bass kernel for the capsule-routing layer (nn_Caps_Layer).

Full inputs: x [32, 512, 768] f32, W [1, 768, 512] f32.
Output: [32, 16, 32] f32.

Strategy: data-parallel over batch across 8 NeuronCores (4 batches/core),
inputs converted to bf16 on the host (halves the HBM traffic; rel-err
budget 2e-2 >> bf16's ~5e-3).

Per core the routing loop is algebraically factored so u_hat [S, N*C]
is never materialized:
    iter0:   m0[(nc)]   = xsum @ W             (xsum = col-sum of x)
    V[d,n]   = sum_c W[d,(n c)] * mnorm[n,c]   (Wt-chunk @ Mblk, ap=16)
    b[s,n]   = x @ V                           (xT-chunk @ V,     ap=16)
    c        = softmax_n(b)
    G[n,d]   = c^T @ x                         (x-chunk @ c,      ap=16)
    m[(nc)]  = diag_n(W^T G)                   (W-chunk @ G^T,    ap=16)
    squash: inv = exp(-0.5 ln(|m|^2 + eps))    (one ACT table: exp+ln)
All routing matmuls keep the tiny capsule dim (16) as the moving side, so
PE streaming cost is ~16 cycles/matmul; the only large PE work is the
x-transposes (needed for the d-major contraction in b = x @ V).
"""
import numpy as np
import concourse.bass as bass
import concourse.mybir as mybir
import concourse.tile as tile
from concourse import bacc
from concourse.bass import ts, ds
from concourse.bass_utils import run_bass_kernel_spmd

F32 = mybir.dt.float32
BF16 = mybir.dt.bfloat16
AF = mybir.ActivationFunctionType
AX = mybir.AxisListType
OP = mybir.AluOpType

NCORES = 8
B, S, D = 32, 512, 768
N, C = 16, 32
NC = N * C            # 512
BL = B // NCORES      # 4 batches per core
EPS = 1e-7
SCN = S // 128        # 4 s-chunks
DCN = D // 128        # 6 d-chunks
KCN = NC // 128       # 4 nc-chunks
ROUTINGS = 3

# const tile column layout (all bf16)
CID = 0               # [128, 128] identity (PE transposes)
CMASK = 128           # [128, 256] diag mask[(nl,c), (b,kc,n)] = (n == 4*kc+nl)
CSEL = 384            # [128, 4]   sel[p, j] = (p//32 == j)
CONE = 388            # [128, 1]   ones
CSELT = 392           # rows 0:4, cols 392:520: selt[j, p] = (p//32 == j)
CONW = 520


def _build_module():
    nc = bacc.Bacc("TRN2", target_bir_lowering=False, num_devices=NCORES)
    X = nc.dram_tensor("x", [BL, S, D], BF16, kind="ExternalInput")
    W = nc.dram_tensor("w", [D, NC], BF16, kind="ExternalInput")
    CON = nc.dram_tensor("consts", [128, CONW], BF16, kind="ExternalInput")
    OUT = nc.dram_tensor("out", [BL, N, C], F32, kind="ExternalOutput")

    cp_flip = [0]

    with tile.TileContext(nc) as tc:
        with (
            tc.tile_pool(name="const", bufs=1) as pc,
            tc.tile_pool(name="rt", bufs=2) as prt,
            tc.tile_pool(name="pmm", bufs=1, space="PSUM") as pmm,
            tc.tile_pool(name="ptr", bufs=3, space="PSUM") as ptr,
        ):
            def cp(dst, src):
                # PSUM->SBUF evacuations alternate DVE/ACT
                if cp_flip[0] % 2 == 0:
                    nc.vector.tensor_copy(dst, src)
                else:
                    nc.scalar.copy(dst, src)
                cp_flip[0] += 1

            # ---- persistent tiles ----
            con = pc.tile([128, CONW], BF16, tag="con")
            wsb = pc.tile([128, DCN, NC], BF16, tag="w")
            wtsb = pc.tile([128, KCN, D], BF16, tag="wt")
            xsumb = pc.tile([128, BL * DCN], BF16, tag="xsum")
            epst = pc.tile([128, 1], F32, tag="eps")
            nc.vector.memset(epst[:], EPS)

            def prefetch_act(func):
                # dummy [1,1] activation hoists the ACT table load early
                dum = prt.tile([1, 1], F32, tag="dum")
                nc.scalar.activation(dum[:], epst[0:1, :], func)

            # consts ride the ACT queue; x batches + W ride the SP queue so
            # the DMA-engine order is con, x0..x3, W.
            nc.scalar.dma_start(con[:], CON[:, :])
            xbs = []
            for b in range(BL):
                xb = pc.tile([128, SCN, D], BF16, tag=f"xb{b}", name=f"xb{b}")
                nc.sync.dma_start(
                    xb[:], X[b, :, :].rearrange("(sc p) d -> p sc d", p=128)
                )
                xbs.append(xb)
            nc.sync.dma_start(
                wsb[:], W[:, :].rearrange("(dc p) n -> p dc n", p=128)
            )
            prefetch_act(AF.Exp)
            prefetch_act(AF.Ln)

            ident = con[:, CID:CID + 128]

            # ---- stage A: xT + xsum per batch ----
            pxs = pmm.tile([128, BL * DCN], F32, tag="sm")
            xts = []
            for b in range(BL):
                xb = xbs[b]
                xt = pc.tile([128, DCN, S], BF16, tag=f"xt{b}", name=f"xt{b}")
                for dc in range(DCN):
                    pxt = ptr.tile([128, S], BF16, tag="tr")
                    for sc in range(SCN):
                        nc.tensor.transpose(
                            pxt[:, ts(sc, 128)],
                            xb[:, sc, ds(dc * 128, 128)],
                            ident,
                        )
                    cp(xt[:, dc, :], pxt[:])
                    for sc in range(SCN):
                        nc.tensor.matmul(
                            pxs[:, ds(b * DCN + dc, 1)],
                            xb[:, sc, ds(dc * 128, 128)],
                            con[:, CONE:CONE + 1],
                            start=(sc == 0),
                            stop=(sc == SCN - 1),
                        )
                cp(xsumb[:, ds(b * DCN, DCN)], pxs[:, ds(b * DCN, DCN)])
                xts.append(xt)

            # ---- WT = W^T chunks (needed from V0 onward) ----
            for kc in range(KCN):
                ptw = ptr.tile([128, S], BF16, tag="tr")
                for dc in range(DCN):
                    half = dc // 4
                    if dc % 4 == 0 and half == 1:
                        cp(wtsb[:, kc, 0:512], ptw[:])
                        ptw = ptr.tile([128, S], BF16, tag="tr")
                    nc.tensor.transpose(
                        ptw[:, ts(dc % 4, 128)],
                        wsb[:, dc, ds(kc * 128, 128)],
                        ident,
                    )
                cp(wtsb[:, kc, 512:768], ptw[:, 0:256])

            # ---- routing ----
            maskr = con[:, CMASK:CMASK + BL * KCN * N]

            def squash(pot, src_cols, it):
                """pot: psum [128, (b kc[ n])] -> returns mnorm tile.
                src_cols=1 for iter0 (pot is [128, (b kc)] = m directly)."""
                small = prt.tile([128, 16], F32, tag="m", name=f"m{it}")
                if src_cols == 1:
                    nc.vector.tensor_copy(small[:], pot[:])
                    m = small
                else:
                    pm = prt.tile([128, BL * KCN * N], F32, tag="pm")
                    nc.vector.tensor_mul(pm[:], pot[:], maskr)
                    nc.vector.tensor_reduce(
                        small[:],
                        pm[:].rearrange("p (g n) -> p g n", g=BL * KCN),
                        axis=AX.X,
                        op=OP.add,
                    )
                    m = small
                sq = prt.tile([128, 16], BF16, tag="sq", name=f"sq{it}")
                nc.scalar.activation(sq[:], m[:], AF.Square)
                pnsq = pmm.tile([128, 16], F32, tag="sm", name=f"nsq{it}")
                nc.tensor.matmul(
                    pnsq[0:4, :],
                    con[:, CSEL:CSEL + 4],
                    sq[:],
                    start=True,
                    stop=True,
                )
                lnv = prt.tile([4, 16], F32, tag="lnv", name=f"lnv{it}")
                nc.scalar.activation(
                    lnv[:], pnsq[0:4, :], AF.Ln, bias=epst[0:4, :]
                )
                rsq = prt.tile([4, 16], BF16, tag="rsq", name=f"rsq{it}")
                nc.scalar.activation(rsq[:], lnv[:], AF.Exp, scale=-0.5)
                pinv = pmm.tile([128, 16], F32, tag="sm2", name=f"pinv{it}")
                nc.tensor.matmul(
                    pinv[:],
                    con[0:4, CSELT:CSELT + 128],
                    rsq[:],
                    start=True,
                    stop=True,
                )
                dt = F32 if it == ROUTINGS - 1 else BF16
                mnorm = prt.tile([128, 16], dt, tag=f"mn{it % 2}",
                                 name=f"mn{it}")
                nc.vector.tensor_mul(mnorm[:], m[:], pinv[:])
                return mnorm

            def v_and_b(mnorm, it):
                """Mblk scatter -> V -> b (psum) for the next iteration."""
                mblk = prt.tile([128, BL * KCN * N], BF16, tag="mblk",
                                name=f"mblk{it}")
                mn_bc = bass.AP(
                    tensor=mnorm.tensor,
                    offset=mnorm.offset,
                    ap=[mnorm.ap[0], [KCN, BL], [1, KCN], [0, N]],
                )
                nc.vector.tensor_mul(
                    mblk[:].rearrange("p (b k n) -> p b k n", b=BL, k=KCN),
                    mn_bc,
                    maskr.rearrange("p (b k n) -> p b k n", b=BL, k=KCN),
                )
                pv = pmm.tile([128, BL * DCN * N], F32, tag="big")
                for b in range(BL):
                    for dc in range(DCN):
                        for kc in range(KCN):
                            nc.tensor.matmul(
                                pv[:, ds((b * DCN + dc) * N, N)],
                                wtsb[:, kc, ds(dc * 128, 128)],
                                mblk[:, ds((b * KCN + kc) * N, N)],
                                start=(kc == 0),
                                stop=(kc == KCN - 1),
                            )
                vsb = prt.tile([128, BL * DCN * N], BF16, tag="vsb")
                nc.scalar.copy(vsb[:], pv[:])
                pb = pmm.tile([128, BL * SCN * N], F32, tag="seq")
                for b in range(BL):
                    for sc in range(SCN):
                        for dc in range(DCN):
                            nc.tensor.matmul(
                                pb[:, ds((b * SCN + sc) * N, N)],
                                xts[b][:, dc, ds(sc * 128, 128)],
                                vsb[:, ds((b * DCN + dc) * N, N)],
                                start=(dc == 0),
                                stop=(dc == DCN - 1),
                            )
                return pb

            # iter 0: uniform routing weights -> m0 = xsum @ W (diag blocks)
            pot0 = pmm.tile([128, BL * KCN], F32, tag="seq")
            for b in range(BL):
                for kc in range(KCN):
                    for dc in range(DCN):
                        nc.tensor.matmul(
                            pot0[:, ds(b * KCN + kc, 1)],
                            wsb[:, dc, ds(kc * 128, 128)],
                            xsumb[:, ds(b * DCN + dc, 1)],
                            start=(dc == 0),
                            stop=(dc == DCN - 1),
                        )
            mnorm = squash(pot0, 1, 0)
            pb = v_and_b(mnorm, 0)

            for it in range(1, ROUTINGS):
                # softmax over n
                expb = prt.tile([128, BL * SCN * N], F32, tag="expb",
                                name=f"expb{it}")
                nc.scalar.activation(expb[:], pb[:], AF.Exp)
                zsum = prt.tile([128, BL * SCN], F32, tag="zsum",
                                name=f"zsum{it}")
                nc.vector.tensor_reduce(
                    zsum[:],
                    expb[:].rearrange("p (g n) -> p g n", g=BL * SCN),
                    axis=AX.X,
                    op=OP.add,
                )
                zrec = prt.tile([128, BL * SCN], F32, tag="zrec",
                                name=f"zrec{it}")
                nc.vector.reciprocal(zrec[:], zsum[:])
                cw = prt.tile([128, BL * SCN * N], BF16, tag="cw",
                              name=f"cw{it}")
                zr_bc = bass.AP(
                    tensor=zrec.tensor,
                    offset=zrec.offset,
                    ap=[zrec.ap[0], [1, BL * SCN], [0, N]],
                )
                nc.vector.tensor_mul(
                    cw[:].rearrange("p (g n) -> p g n", g=BL * SCN),
                    expb[:].rearrange("p (g n) -> p g n", g=BL * SCN),
                    zr_bc,
                )
                # G^T[d, n] per (b, dc)
                pg = pmm.tile([128, BL * DCN * N], F32, tag="big",
                              name=f"gp{it}")
                for b in range(BL):
                    for dc in range(DCN):
                        for sc in range(SCN):
                            nc.tensor.matmul(
                                pg[:, ds((b * DCN + dc) * N, N)],
                                xbs[b][:, sc, ds(dc * 128, 128)],
                                cw[:, ds((b * SCN + sc) * N, N)],
                                start=(sc == 0),
                                stop=(sc == SCN - 1),
                            )
                gsb = prt.tile([128, BL * DCN * N], BF16, tag="gsb",
                               name=f"gsb{it}")
                nc.scalar.copy(gsb[:], pg[:])
                # outT[(nc), n] per (b, kc)
                pot = pmm.tile([128, BL * KCN * N], F32, tag="seq",
                               name=f"potp{it}")
                for b in range(BL):
                    for kc in range(KCN):
                        for dc in range(DCN):
                            nc.tensor.matmul(
                                pot[:, ds((b * KCN + kc) * N, N)],
                                wsb[:, dc, ds(kc * 128, 128)],
                                gsb[:, ds((b * DCN + dc) * N, N)],
                                start=(dc == 0),
                                stop=(dc == DCN - 1),
                            )
                mnorm = squash(pot, N, it)
                if it < ROUTINGS - 1:
                    pb = v_and_b(mnorm, it)

            # final output: mnorm [128=(nl,c), (b kc)] f32 -> OUT[b, n, c]
            nc.sync.dma_start(
                OUT.rearrange("b (kc nl) c -> (nl c) (b kc)", kc=KCN, nl=4),
                mnorm[:],
            )

    nc.compile()
    return nc


def _make_consts():
    import ml_dtypes
    con = np.zeros((128, CONW), dtype=np.float32)
    con[:, CID:CID + 128] = np.eye(128, dtype=np.float32)
    p = np.arange(128)
    for b in range(BL):
        for kc in range(KCN):
            for n in range(N):
                con[:, CMASK + (b * KCN + kc) * N + n] = (n == 4 * kc + p // 32)
    for j in range(4):
        con[:, CSEL + j] = (p // 32 == j)
    con[:, CONE] = 1.0
    for j in range(4):
        con[j, CSELT:CSELT + 128] = (p // 32 == j)
    return con.astype(ml_dtypes.bfloat16)


_NC_CACHE = []


def kernel(x: np.ndarray, W: np.ndarray) -> np.ndarray:
    import ml_dtypes
    assert x.shape == (B, S, D) and W.shape == (1, D, NC)
    if not _NC_CACHE:
        _NC_CACHE.append(_build_module())
    nc = _NC_CACHE[0]
    con = _make_consts()
    w2 = np.ascontiguousarray(W[0]).astype(ml_dtypes.bfloat16)
    xb = x.astype(ml_dtypes.bfloat16)
    in_maps = []
    for i in range(NCORES):
        m = {
            "x": np.ascontiguousarray(xb[i * BL:(i + 1) * BL]),
            "w": w2,
            "consts": con,
        }
        in_maps.append(m)
    res = run_bass_kernel_spmd(nc, in_maps, list(range(NCORES)))
    out = np.concatenate([res.results[i]["out"] for i in range(NCORES)], axis=0)
    return out.astype(np.float32)


# revision 7
# speedup vs baseline: 1.8142x; 1.0333x over previous
"""Trainium2 Bass kernel for the capsule-routing layer (nn_Caps_Layer).

Full inputs: x [32, 512, 768] f32, W [1, 768, 512] f32.
Output: [32, 16, 32] f32.

Strategy: data-parallel over batch across 8 NeuronCores (4 batches/core),
inputs converted to bf16 on the host (halves the HBM traffic; rel-err
budget 2e-2 >> bf16's ~5e-3).

Per core the routing loop is algebraically factored so u_hat [S, N*C]
is never materialized:
    iter0:   m0[(nc)]   = xsum @ W             (xsum = col-sum of x)
    V[d,n]   = sum_c W[d,(n c)] * mnorm[n,c]   (Wt-chunk @ Mblk, ap=16)
    b[s,n]   = x @ V                           (xT-chunk @ V,     ap=16)
    c        = softmax_n(b)
    G[n,d]   = c^T @ x                         (x-chunk @ c,      ap=16)
    m[(nc)]  = diag_n(W^T G)                   (W-chunk @ G^T,    ap=16)
    squash: inv = exp(-0.5 ln(|m|^2 + eps))    (one ACT table: exp+ln)
All routing matmuls keep the tiny capsule dim (16) as the moving side, so
PE streaming cost is ~16 cycles/matmul; the only large PE work is the
x-transposes (needed for the d-major contraction in b = x @ V).
"""
import numpy as np
import concourse.bass as bass
import concourse.mybir as mybir
import concourse.tile as tile
from concourse import bacc
from concourse.bass import ts, ds
from concourse.bass_utils import run_bass_kernel_spmd

F32 = mybir.dt.float32
U32 = mybir.dt.uint32
BF16 = mybir.dt.bfloat16
AF = mybir.ActivationFunctionType
AX = mybir.AxisListType
OP = mybir.AluOpType

NCORES = 8
B, S, D = 32, 512, 768
N, C = 16, 32
NC = N * C            # 512
BL = B // NCORES      # 4 batches per core
EPS = 1e-7
SCN = S // 128        # 4 s-chunks
DCN = D // 128        # 6 d-chunks
KCN = NC // 128       # 4 nc-chunks
ROUTINGS = 3

# const tile column layout (all bf16)
CID = 0               # [128, 128] identity (PE transposes)
CMASK = 128           # [128, 256] diag mask[(nl,c), (b,kc,n)] = (n == 4*kc+nl)
CSEL = 384            # [128, 4]   sel[p, j] = (p//32 == j)
CONE = 388            # [128, 1]   ones
CSELT = 392           # rows 0:4, cols 392:520: selt[j, p] = (p//32 == j)
CONW = 520


def _build_module():
    nc = bacc.Bacc("TRN2", target_bir_lowering=False, num_devices=NCORES)
    X = nc.dram_tensor("x", [BL, S, D], BF16, kind="ExternalInput")
    W = nc.dram_tensor("w", [D, NC], BF16, kind="ExternalInput")
    CON = nc.dram_tensor("consts", [128, CONW], BF16, kind="ExternalInput")
    OUT = nc.dram_tensor("out", [BL, N, C], F32, kind="ExternalOutput")

    cp_flip = [0]

    with tile.TileContext(nc) as tc:
        with (
            tc.tile_pool(name="const", bufs=1) as pc,
            tc.tile_pool(name="rt", bufs=2) as prt,
            tc.tile_pool(name="pmm", bufs=1, space="PSUM") as pmm,
            tc.tile_pool(name="ptr", bufs=3, space="PSUM") as ptr,
        ):
            def cp(dst, src):
                # PSUM->SBUF evacuations rotate DVE/ACT/DVE/Pool
                r = cp_flip[0] % 3
                if r in (0, 2):
                    nc.vector.tensor_copy(dst, src)
                else:
                    nc.scalar.copy(dst, src)
                cp_flip[0] += 1

            # ---- persistent tiles ----
            con = pc.tile([128, CONW], BF16, tag="con")
            wsb = pc.tile([128, DCN, NC], BF16, tag="w")
            wtsb = pc.tile([128, KCN, D], BF16, tag="wt")
            xsumb = pc.tile([128, BL * DCN], BF16, tag="xsum")
            epst = pc.tile([128, 1], F32, tag="eps")
            nc.vector.memset(epst[:], EPS)
            magict = pc.tile([128, 16], U32, tag="magic")
            nc.vector.memset(magict[:], 0x5F3759DF)

            def prefetch_act(func):
                # dummy [1,1] activation hoists the ACT table load early
                dum = prt.tile([1, 1], F32, tag="dum")
                nc.scalar.activation(dum[:], epst[0:1, :], func)

            # consts ride the ACT queue; x batches + W ride the SP queue.
            # W sits between x1 and x2 so WT transposes clear the PE early;
            # the last batch arrives in s-chunks so stage A can track it.
            nc.scalar.dma_start(con[:], CON[:, :])
            xbs = [pc.tile([128, SCN, D], BF16, tag=f"xb{b}", name=f"xb_{b}")
                   for b in range(BL)]
            for b in range(2):
                nc.sync.dma_start(
                    xbs[b][:],
                    X[b, :, :].rearrange("(sc p) d -> p sc d", p=128),
                )
            nc.sync.dma_start(
                wsb[:], W[:, :].rearrange("(dc p) n -> p dc n", p=128)
            )
            nc.sync.dma_start(
                xbs[2][:], X[2, :, :].rearrange("(sc p) d -> p sc d", p=128)
            )
            for sc in range(SCN):
                nc.sync.dma_start(
                    xbs[3][:, sc, :], X[3, ds(sc * 128, 128), :]
                )
            prefetch_act(AF.Exp)

            ident = con[:, CID:CID + 128]

            # ---- WT = W^T chunks (early; W lands mid-DMA) ----
            for kc in range(KCN):
                ptw = ptr.tile([128, S], BF16, tag="tr")
                for dc in range(DCN):
                    half = dc // 4
                    if dc % 4 == 0 and half == 1:
                        cp(wtsb[:, kc, 0:512], ptw[:])
                        ptw = ptr.tile([128, S], BF16, tag="tr")
                    nc.tensor.transpose(
                        ptw[:, ts(dc % 4, 128)],
                        wsb[:, dc, ds(kc * 128, 128)],
                        ident,
                    )
                cp(wtsb[:, kc, 512:768], ptw[:, 0:256])

            # ---- stage A: xT + xsum per batch ----
            pxs = pmm.tile([128, BL * DCN], F32, tag="sm")
            xts = []
            for b in range(BL):
                xb = xbs[b]
                xt = pc.tile([128, DCN, S], BF16, tag=f"xt{b}", name=f"xt{b}")
                for dc in range(DCN):
                    pxt = ptr.tile([128, S], BF16, tag="tr")
                    for sc in range(SCN):
                        nc.tensor.transpose(
                            pxt[:, ts(sc, 128)],
                            xb[:, sc, ds(dc * 128, 128)],
                            ident,
                        )
                    cp(xt[:, dc, :], pxt[:])
                    for sc in range(SCN):
                        nc.tensor.matmul(
                            pxs[:, ds(b * DCN + dc, 1)],
                            xb[:, sc, ds(dc * 128, 128)],
                            con[:, CONE:CONE + 1],
                            start=(sc == 0),
                            stop=(sc == SCN - 1),
                        )
                cp(xsumb[:, ds(b * DCN, DCN)], pxs[:, ds(b * DCN, DCN)])
                xts.append(xt)

            # ---- routing ----
            maskr = con[:, CMASK:CMASK + BL * KCN * N]

            def squash(pot, src_cols, it):
                """pot: psum [128, (b kc[ n])] -> returns mnorm tile.
                src_cols=1 for iter0 (pot is [128, (b kc)] = m directly)."""
                small = prt.tile([128, 16], F32, tag="m", name=f"m{it}")
                if src_cols == 1:
                    nc.vector.tensor_copy(small[:], pot[:])
                    m = small
                else:
                    pm = prt.tile([128, BL * KCN * N], F32, tag="pm")
                    nc.vector.tensor_mul(pm[:], pot[:], maskr)
                    nc.vector.tensor_reduce(
                        small[:],
                        pm[:].rearrange("p (g n) -> p g n", g=BL * KCN),
                        axis=AX.X,
                        op=OP.add,
                    )
                    m = small
                sq = prt.tile([128, 16], BF16, tag="sq", name=f"sq{it}")
                nc.vector.tensor_mul(sq[:], m[:], m[:])
                pnsq = pmm.tile([128, 16], F32, tag="sm", name=f"nsq{it}")
                nc.tensor.matmul(
                    pnsq[0:4, :],
                    con[:, CSEL:CSEL + 4],
                    sq[:],
                    start=True,
                    stop=True,
                )
                # rsqrt on DVE only (bit trick + 1 Newton step); keeps the
                # ACT table pinned to Exp for the whole kernel
                nsqs = prt.tile([4, 16], F32, tag="nsqs", name=f"nsqs{it}")
                nc.vector.tensor_scalar_add(nsqs[:], pnsq[0:4, :], EPS)
                y0u = prt.tile([4, 16], U32, tag="y0u", name=f"y0u{it}")
                nc.vector.tensor_scalar(
                    y0u[:], nsqs[:].bitcast(U32), 1, None,
                    OP.logical_shift_right,
                )
                nc.vector.tensor_sub(y0u[:], magict[0:4, :], y0u[:])
                y0f = y0u[:].bitcast(F32)
                t1 = prt.tile([4, 16], F32, tag="nt1", name=f"nt1{it}")
                nc.vector.tensor_mul(t1[:], nsqs[:], y0f)
                nc.vector.tensor_mul(t1[:], t1[:], y0f)
                nc.vector.tensor_scalar(t1[:], t1[:], -0.5, 1.5, OP.mult,
                                        OP.add)
                rsq = prt.tile([4, 16], BF16, tag="rsq", name=f"rsq{it}")
                nc.vector.tensor_mul(rsq[:], y0f, t1[:])
                pinv = pmm.tile([128, 16], F32, tag="sm2", name=f"pinv{it}")
                nc.tensor.matmul(
                    pinv[:],
                    con[0:4, CSELT:CSELT + 128],
                    rsq[:],
                    start=True,
                    stop=True,
                )
                dt = F32 if it == ROUTINGS - 1 else BF16
                mnorm = prt.tile([128, 16], dt, tag=f"mn{it % 2}",
                                 name=f"mn{it}")
                nc.vector.tensor_mul(mnorm[:], m[:], pinv[:])
                return mnorm

            def v_and_b(mnorm, it):
                """Mblk scatter -> V -> b (psum) for the next iteration."""
                mblk = prt.tile([128, BL * KCN * N], BF16, tag="mblk",
                                name=f"mblk{it}")
                mn_bc = bass.AP(
                    tensor=mnorm.tensor,
                    offset=mnorm.offset,
                    ap=[mnorm.ap[0], [KCN, BL], [1, KCN], [0, N]],
                )
                nc.vector.tensor_mul(
                    mblk[:].rearrange("p (b k n) -> p b k n", b=BL, k=KCN),
                    mn_bc,
                    maskr.rearrange("p (b k n) -> p b k n", b=BL, k=KCN),
                )
                pv = pmm.tile([128, BL * DCN * N], F32, tag="big")
                for b in range(BL):
                    for dc in range(DCN):
                        for kc in range(KCN):
                            nc.tensor.matmul(
                                pv[:, ds((b * DCN + dc) * N, N)],
                                wtsb[:, kc, ds(dc * 128, 128)],
                                mblk[:, ds((b * KCN + kc) * N, N)],
                                start=(kc == 0),
                                stop=(kc == KCN - 1),
                            )
                vsb = prt.tile([128, BL * DCN * N], BF16, tag="vsb")
                nc.scalar.copy(vsb[:], pv[:])
                pb = pmm.tile([128, BL * SCN * N], F32, tag="seq")
                for b in range(BL):
                    for sc in range(SCN):
                        for dc in range(DCN):
                            nc.tensor.matmul(
                                pb[:, ds((b * SCN + sc) * N, N)],
                                xts[b][:, dc, ds(sc * 128, 128)],
                                vsb[:, ds((b * DCN + dc) * N, N)],
                                start=(dc == 0),
                                stop=(dc == DCN - 1),
                            )
                return pb

            # iter 0: uniform routing weights -> m0 = xsum @ W (diag blocks)
            pot0 = pmm.tile([128, BL * KCN], F32, tag="seq")
            for b in range(BL):
                for kc in range(KCN):
                    for dc in range(DCN):
                        nc.tensor.matmul(
                            pot0[:, ds(b * KCN + kc, 1)],
                            wsb[:, dc, ds(kc * 128, 128)],
                            xsumb[:, ds(b * DCN + dc, 1)],
                            start=(dc == 0),
                            stop=(dc == DCN - 1),
                        )
            mnorm = squash(pot0, 1, 0)
            pb = v_and_b(mnorm, 0)

            for it in range(1, ROUTINGS):
                # softmax over n
                expb = prt.tile([128, BL * SCN * N], F32, tag="expb",
                                name=f"expb{it}")
                nc.scalar.activation(expb[:], pb[:], AF.Exp)
                zsum = prt.tile([128, BL * SCN], F32, tag="zsum",
                                name=f"zsum{it}")
                nc.vector.tensor_reduce(
                    zsum[:],
                    expb[:].rearrange("p (g n) -> p g n", g=BL * SCN),
                    axis=AX.X,
                    op=OP.add,
                )
                zrec = prt.tile([128, BL * SCN], F32, tag="zrec",
                                name=f"zrec{it}")
                nc.vector.reciprocal(zrec[:], zsum[:])
                cw = prt.tile([128, BL * SCN * N], BF16, tag="cw",
                              name=f"cw{it}")
                zr_bc = bass.AP(
                    tensor=zrec.tensor,
                    offset=zrec.offset,
                    ap=[zrec.ap[0], [1, BL * SCN], [0, N]],
                )
                nc.vector.tensor_mul(
                    cw[:].rearrange("p (g n) -> p g n", g=BL * SCN),
                    expb[:].rearrange("p (g n) -> p g n", g=BL * SCN),
                    zr_bc,
                )
                # G^T[d, n] per (b, dc)
                pg = pmm.tile([128, BL * DCN * N], F32, tag="big",
                              name=f"gp{it}")
                for b in range(BL):
                    for dc in range(DCN):
                        for sc in range(SCN):
                            nc.tensor.matmul(
                                pg[:, ds((b * DCN + dc) * N, N)],
                                xbs[b][:, sc, ds(dc * 128, 128)],
                                cw[:, ds((b * SCN + sc) * N, N)],
                                start=(sc == 0),
                                stop=(sc == SCN - 1),
                            )
                gsb = prt.tile([128, BL * DCN * N], BF16, tag="gsb",
                               name=f"gsb{it}")
                nc.scalar.copy(gsb[:], pg[:])
                # outT[(nc), n] per (b, kc)
                pot = pmm.tile([128, BL * KCN * N], F32, tag="seq",
                               name=f"potp{it}")
                for b in range(BL):
                    for kc in range(KCN):
                        for dc in range(DCN):
                            nc.tensor.matmul(
                                pot[:, ds((b * KCN + kc) * N, N)],
                                wsb[:, dc, ds(kc * 128, 128)],
                                gsb[:, ds((b * DCN + dc) * N, N)],
                                start=(dc == 0),
                                stop=(dc == DCN - 1),
                            )
                mnorm = squash(pot, N, it)
                if it < ROUTINGS - 1:
                    pb = v_and_b(mnorm, it)

            # final output: mnorm [128=(nl,c), (b kc)] f32 -> OUT[b, n, c]
            nc.sync.dma_start(
                OUT.rearrange("b (kc nl) c -> (nl c) (b kc)", kc=KCN, nl=4),
                mnorm[:],
            )

    nc.compile()
    return nc


def _make_consts():
    import ml_dtypes
    con = np.zeros((128, CONW), dtype=np.float32)
    con[:, CID:CID + 128] = np.eye(128, dtype=np.float32)
    p = np.arange(128)
    for b in range(BL):
        for kc in range(KCN):
            for n in range(N):
                con[:, CMASK + (b * KCN + kc) * N + n] = (n == 4 * kc + p // 32)
    for j in range(4):
        con[:, CSEL + j] = (p // 32 == j)
    con[:, CONE] = 1.0
    for j in range(4):
        con[j, CSELT:CSELT + 128] = (p // 32 == j)
    return con.astype(ml_dtypes.bfloat16)


_NC_CACHE = []


def kernel(x: np.ndarray, W: np.ndarray) -> np.ndarray:
    import ml_dtypes
    assert x.shape == (B, S, D) and W.shape == (1, D, NC)
    if not _NC_CACHE:
        _NC_CACHE.append(_build_module())
    nc = _NC_CACHE[0]
    con = _make_consts()
    w2 = np.ascontiguousarray(W[0]).astype(ml_dtypes.bfloat16)
    xb = x.astype(ml_dtypes.bfloat16)
    in_maps = []
    for i in range(NCORES):
        m = {
            "x": np.ascontiguousarray(xb[i * BL:(i + 1) * BL]),
            "w": w2,
            "consts": con,
        }
        in_maps.append(m)
    res = run_bass_kernel_spmd(nc, in_maps, list(range(NCORES)))
    out = np.concatenate([res.results[i]["out"] for i in range(NCORES)], axis=0)
    return out.astype(np.float32)
